# revision 1
# baseline (speedup 1.0000x reference)
"""Trainium2 Bass kernel for CubPL2d persistence-landscape problem.

Computes, for full inputs
    x:         [128, 64, 64, 64] f32
    birth_idx: [128, 64, 128] int
    death_idx: [128, 64, 128] int
    pair_dim:  [128, 64, 128] int
the output [128, 64, 2, 2, 32] f32:
    tri[b,c,p,t] = max(min(t_seq[t] - x[b,c,birth], x[b,c,death] - t_seq[t]), 0)
    out[b,c,d,k,t] = k-th largest over p of (tri where pair_dim==d else 0)

Sharding: pure data-parallel over batch dim B across 8 cores (16 batches each).

Per-core algorithm (BC = 16*64 = 1024 (b,c) rows, blocks of 128 rows):
  - stream x rows into SBUF at line rate
  - on-chip gather of births/deaths via GPSIMD ap_gather: each 16-partition
    group gathers the interleaved union of its rows' indices, so channel ch's
    own values land at columns == ch (mod 16); extracted by a per-partition
    16x256 transpose on the scalar engine plus a DRAM round-trip whose
    read-back access pattern selects each partition's own residue row
  - triangle construction on the vector engine (fp16, 2x mode) with
    broadcast access patterns; relu deferred to the very end (relu is
    monotone, so it commutes with top-k)
  - per (dim, t) top-2 over pairs via InstMax (exact top-8 per partition row)
"""

import numpy as np

import concourse.bass as bass
import concourse.bacc as bacc
import concourse.mybir as mybir
from concourse.bass_types import AP
from concourse.tile import TileContext
from concourse.bass_utils import run_bass_kernel_spmd

T_MIN, T_MAX = 0.03, 0.34
STEPS = 32
K_MAX = 2
N_DIMS = 2
B, C, H, W = 128, 64, 64, 64
P = 128
HW = H * W
N_CORES = 8
B_LOC = B // N_CORES  # 16
BC_FULL = B_LOC * C  # 1024 (b,c) rows per core

F32 = mybir.dt.float32
F16 = mybir.dt.float16
I32 = mybir.dt.int32
I16 = mybir.dt.int16
AF = mybir.ActivationFunctionType
ALU = mybir.AluOpType

COMPUTE_DT = F16  # fp16 keeps ~3.5 decimal digits; output scale ~0.34


def build_nc(bc: int = BC_FULL, cdt=COMPUTE_DT, repeat: int = 1,
             ablate: frozenset = frozenset()) -> bass.Bass:
    """Build the single-core Bass program for a shard with `bc` (b,c) rows.

    repeat > 1 wraps the whole block loop in a hardware For loop that redoes
    the (idempotent) computation `repeat` times — benchmarking only.
    ablate: subset of {"gather", "bounce", "construct", "max"} — skip those
    stages (outputs become garbage; timing-bisection only).
    """
    assert bc % 128 == 0
    nb = bc // 128
    tstep = (T_MAX - T_MIN) / (STEPS - 1)

    nc = bacc.Bacc(None, target_bir_lowering=False)
    x_t = nc.dram_tensor("x", [bc, HW], F32, kind="ExternalInput")
    # birth_idx / death_idx / pair_dim packed host-side into one int16 tensor
    idx_t = nc.dram_tensor("idx3", [bc, 3 * P], I16, kind="ExternalInput")
    out_t = nc.dram_tensor("out", [bc, N_DIMS * K_MAX * STEPS], F32,
                           kind="ExternalOutput")
    # DRAM bounce buffer for the per-residue assembly of gathered values
    sb_t = nc.dram_tensor("s_bounce", [bc, 16 * 2 * P], F16)

    with TileContext(nc) as tc:
        with (
            tc.tile_pool(name="const", bufs=1) as cpool,
            tc.tile_pool(name="xrows", bufs=2) as xpool,
            tc.tile_pool(name="idx", bufs=3) as ipool,
            tc.tile_pool(name="small", bufs=3) as spool,
            tc.tile_pool(name="big", bufs=2) as bpool,
        ):
            # t_rep tile [128, STEPS, P]: t value replicated along p, so every
            # operand of the big tensor_tensor ops is packed in its last dim
            # (required for the DVE 2x_1p fp16 mode).
            t_rep = cpool.tile([128, STEPS, P], cdt)
            nc.gpsimd.iota(t_rep[:, :, :], pattern=[[1, STEPS], [0, P]],
                           base=0, channel_multiplier=0,
                           allow_small_or_imprecise_dtypes=True)
            nc.scalar.activation(t_rep[:, :, :], t_rep[:, :, :], AF.Copy,
                                 bias=float(T_MIN), scale=float(tstep))

            import contextlib
            loop_cm = (tc.For_i(0, repeat) if repeat > 1
                       else contextlib.nullcontext())
            with loop_cm:
              for blk in range(nb):
                r0 = blk * 128
                xrow = xpool.tile([128, HW], F32, tag="xrow")
                nc.sync.dma_start(out=xrow[:, :], in_=x_t[r0:r0 + 128, :])
                idx3 = ipool.tile([128, 3 * P], I16, tag="idx3")
                nc.sync.dma_start(out=idx3[:, :], in_=idx_t[r0:r0 + 128, :])
                pdim = idx3[:, 2 * P:3 * P]

                # on-chip gather: each 16-partition group's index list is the
                # interleave of its 16 rows' (birth||death) indices; every
                # channel of the group gathers the whole union from its own
                # x row, its own values sitting at columns == ch (mod 16)
                oic = xpool.tile([128, HW], F32, tag="oic")
                if "gather" in ablate:
                    nc.gpsimd.memset(oic[:, 0:16], 0.25)
                else:
                  nc.gpsimd.ap_gather(
                    out_ap=oic[:, :].rearrange("p (n d) -> p n d", d=1),
                    in_ap=xrow[:, :].rearrange("p (n d) -> p n d", d=1),
                    idxs_ap=idx3[:, 0:2 * P],
                    channels=128,
                    num_elems=HW,
                    d=1,
                    num_idxs=2 * P * 16,
                )
                # reorder on scalar engine (+ f32 -> f16):
                #   S[ch, j, s] = oic[ch, s*16 + j]
                # then bounce S through DRAM; the read-back AP walks (g, j, s)
                # affinely so partition ch = 16g+j receives its own residue
                # row S[ch, ch%16, :] as one contiguous 512B run.
                S = spool.tile([128, 16, 2 * P], F16, tag="S")
                gat16 = spool.tile([128, 2 * P], cdt, tag="gat16")
                if "bounce" in ablate:
                    nc.vector.memset(S[:, 0, 0:8], 0.25)
                    nc.vector.memset(gat16[:, :], 0.25)
                if "bounce" not in ablate:
                    oic_T = AP(oic[:, :].tensor, oic[:, :].offset,
                               [[HW, 128], [1, 16], [16, 2 * P]])
                    nc.scalar.copy(S[:, :, :], oic_T)
                    nc.sync.dma_start(out=sb_t[r0:r0 + 128, :],
                                      in_=S[:, :, :])
                    sb_flat = sb_t[:, :].rearrange("a b -> (a b)")
                    stride_g = 16 * 16 * 2 * P  # 16 rows of S per group
                    stride_j = 16 * 2 * P + 2 * P  # next part + own residue
                    src = AP(sb_flat.tensor, r0 * 16 * 2 * P,
                             [[stride_g, 8], [stride_j, 16], [1, 2 * P]])
                    nc.sync.dma_start(out=gat16[:, :], in_=src)

                # dim-0 mask as 0/1 in compute dtype
                m0 = spool.tile([128, P], cdt, tag="m0")
                nc.gpsimd.tensor_scalar(m0[:, :], pdim, 0, None,
                                        op0=ALU.is_equal)

                births = gat16[:, :P]
                deaths = gat16[:, P:]
                b_b = births.rearrange("p (t q) -> p t q", t=1) \
                            .broadcast_to([128, STEPS, P])
                d_b = deaths.rearrange("p (t q) -> p t q", t=1) \
                            .broadcast_to([128, STEPS, P])
                m0_b = m0[:, :].rearrange("p (t q) -> p t q", t=1) \
                               .broadcast_to([128, STEPS, P])

                # u = t - birth ; v = death - t ; tri = min(u, v)  (no relu)
                u3 = bpool.tile([128, STEPS, P], cdt, tag="u3")
                v3 = bpool.tile([128, STEPS, P], cdt, tag="v3")
                l0 = bpool.tile([128, STEPS, P], cdt, tag="l0")
                if "construct" in ablate:
                    nc.vector.memset(u3[:, 0, 0:8], 0.25)
                    nc.vector.memset(v3[:, 0, 0:8], 0.25)
                    nc.vector.memset(l0[:, 0, 0:8], 0.25)
                if "construct" not in ablate:
                  nc.vector.tensor_tensor(out=u3[:, :, :], in0=t_rep[:, :, :],
                                        in1=b_b, op=ALU.subtract)
                  nc.vector.tensor_tensor(out=v3[:, :, :], in0=d_b,
                                          in1=t_rep[:, :, :], op=ALU.subtract)
                  nc.vector.tensor_tensor(out=u3[:, :, :], in0=u3[:, :, :],
                                          in1=v3[:, :, :], op=ALU.min)
                  # land0 = tri * m0 ; land1 = tri - land0
                  nc.vector.tensor_tensor(out=l0[:, :, :], in0=u3[:, :, :],
                                          in1=m0_b, op=ALU.mult)
                  nc.vector.tensor_tensor(out=u3[:, :, :], in0=u3[:, :, :],
                                          in1=l0[:, :, :], op=ALU.subtract)

                # top-8 over pairs per (dim, t); keep first two later
                top0 = spool.tile([128, STEPS, 8], cdt, tag="top0")
                top1 = spool.tile([128, STEPS, 8], cdt, tag="top1")
                if "max" in ablate:
                    nc.vector.memset(top0[:, 0, :], 0.25)
                    nc.vector.memset(top1[:, 0, :], 0.25)
                if "max" not in ablate:
                  for t in range(STEPS):
                    nc.vector.max(out=top0[:, t, :], in_=l0[:, t, :])
                    nc.vector.max(out=top1[:, t, :], in_=u3[:, t, :])

                # out row layout: (d, k, t); relu applied here
                ot = spool.tile([128, N_DIMS * K_MAX * STEPS], F32, tag="ot")
                for d, top in ((0, top0), (1, top1)):
                    for k in range(K_MAX):
                        s = (d * K_MAX + k) * STEPS
                        nc.scalar.activation(ot[:, s:s + STEPS], top[:, :, k],
                                             AF.Relu)
                nc.sync.dma_start(out=out_t[r0:r0 + 128, :], in_=ot[:, :])

    nc.compile()
    return nc


_NC_CACHE: dict = {}


def _get_nc(bc: int) -> bass.Bass:
    if bc not in _NC_CACHE:
        _NC_CACHE[bc] = build_nc(bc)
    return _NC_CACHE[bc]


def make_in_maps(x, birth_idx, death_idx, pair_dim):
    x = np.asarray(x, dtype=np.float32)
    idx3 = np.stack([
        np.asarray(birth_idx).reshape(B, C, P).astype(np.int16),
        np.asarray(death_idx).reshape(B, C, P).astype(np.int16),
        np.asarray(pair_dim).reshape(B, C, P).astype(np.int16),
    ], axis=2)  # [B, C, 3, P]
    in_maps = []
    for core in range(N_CORES):
        b0, b1 = core * B_LOC, (core + 1) * B_LOC
        in_maps.append({
            "x": np.ascontiguousarray(x[b0:b1].reshape(BC_FULL, HW)),
            "idx3": np.ascontiguousarray(
                idx3[b0:b1].reshape(BC_FULL, 3 * P)),
        })
    return in_maps


def kernel(x, birth_idx, death_idx, pair_dim):
    x = np.asarray(x, dtype=np.float32)
    assert x.shape == (B, C, H, W)
    nc = _get_nc(BC_FULL)
    in_maps = make_in_maps(x, birth_idx, death_idx, pair_dim)
    res = run_bass_kernel_spmd(nc, in_maps, core_ids=list(range(N_CORES)))
    outs = [
        res.results[c]["out"].reshape(B_LOC, C, N_DIMS, K_MAX, STEPS)
        for c in range(N_CORES)
    ]
    return np.concatenate(outs, axis=0).astype(np.float32)

